# revision 18
# baseline (speedup 1.0000x reference)
"""CLIPtrase recalibration kernel for Trainium2 (Bass/Tile).

Per-batch computation (B=8, one batch element per NeuronCore):
    x  : (2304, 768) f32
    xn = x / ||x||_row
    S  = xn @ xn.T              (cosine correlation, symmetric)
    W  = softmax(S / 0.05, axis=-1)
    out = 0.5 * x + 0.5 * (W @ x)

Layout strategy (keys-on-partition, symmetric-S reuse, xbar transposes):
    - xT: xn transposed into [j-tile][d, n-local] bf16 layout via DMA xbar
      transposes (dma_start_transpose), so the tensor engine does matmuls
      only. Both QK operands come from xT.
    - S^T[a-keys(part), b-queries(free)] is computed per 512-query megablock
      m, but only for key tiles a <= amax(m) (upper parallelogram). exp is
      fused on ACT: E = exp(20*S - 20); the softmax row max is exactly
      cos(i,i)=1 (scaled: 20), so bias=-20 replaces max-subtraction. All E
      blocks stay resident in SBUF (bf16, ~49KB/partition).
    - E is symmetric, so the PV stationary blocks missing from the upper
      parallelogram are DMA-xbar transposes of stored blocks instead of
      full recomputation -- a ~40% QK matmul saving.
    - PV rhs = [x | 1] bf16: the ones column makes PSUM col 768 accumulate
      the softmax denominator alongside recal in cols 0:768.
"""

import sys

sys.path.insert(0, "/opt/trn_rl_repo")

import numpy as np

import concourse.bass as bass
import concourse.mybir as mybir
from concourse import bacc
from concourse.bass_utils import run_bass_kernel_spmd
from concourse.tile import TileContext

F32 = mybir.dt.float32
BF16 = mybir.dt.bfloat16

B = 8
H = 48
W = 48
N = H * W          # 2304
D = 768
P = 128
NT = N // P        # 18 row tiles
DT = D // P        # 6 feature tiles
TEMP_INV = 20.0    # 1 / 0.05
# query megablocks: (start, width)
MEGAS = [(0, 512), (512, 512), (1024, 512), (1536, 512), (2048, 256)]

_CACHED = {}


def build_program():
    nc = bacc.Bacc()
    x_in = nc.declare_dram_parameter("x", [N, D], F32, isOutput=False)
    out_dram = nc.declare_dram_parameter("out", [N, D], F32, isOutput=True)

    mega_of_block = {}
    amax = []
    for mi, (q0, qw) in enumerate(MEGAS):
        for b in range(q0 // P, (q0 + qw) // P):
            mega_of_block[b] = mi
        amax.append((q0 + qw) // P - 1)

    with TileContext(nc) as tc:
        with (
            tc.tile_pool(name="persist", bufs=1) as persist,
            tc.tile_pool(name="work", bufs=3) as work,
            tc.tile_pool(name="estore", bufs=1) as estore,
            tc.tile_pool(name="psS", bufs=2, space="PSUM") as psum_s,
            tc.tile_pool(name="psPV", bufs=3, space="PSUM") as psum_pv,
        ):
            # persistent tensors
            x_full = persist.tile([P, NT, D], F32)      # raw x, resident
            # xn^T, j-major: xT[p, j, d, c] = xn[j*128+c, d*128+p]
            xT = persist.tile([P, NT, DT, P], BF16)
            x_aug = persist.tile([P, NT, D + 1], BF16)  # [x | 1] by row tile
            nc.vector.memset(x_aug[:, :, D : D + 1], 1.0)
            zero_bias = persist.tile([P, 1], F32)
            nc.vector.memset(zero_bias, 0.0)
            negtemp = persist.tile([P, 1], F32)
            nc.vector.memset(negtemp, -TEMP_INV)

            # E storage: per mega m, E^T[a, b-cols] for a <= amax(m)
            e_tiles = [
                estore.tile(
                    [P, amax[mi] + 1, qw], BF16, tag=f"E{mi}", name=f"E{mi}"
                )
                for mi, (q0, qw) in enumerate(MEGAS)
            ]

            # ---- Phase 1: load, row norms, xn, xbar-transpose to xT ----
            for j in range(NT):
                xt = x_full[:, j, :]
                # two half-width DMAs on different queues halve arrival latency
                nc.sync.dma_start(
                    out=x_full[:, j, 0 : D // 2],
                    in_=x_in[j * P : (j + 1) * P, 0 : D // 2],
                )
                nc.sync.dma_start(
                    out=x_full[:, j, D // 2 : D],
                    in_=x_in[j * P : (j + 1) * P, D // 2 : D],
                )

                # sum of squares in one pass: (x*1)*x with accumulate
                scratch = work.tile([P, D], BF16, tag="scratch")
                ssum = work.tile([P, 1], F32, tag="ssum")
                nc.vector.scalar_tensor_tensor(
                    scratch,
                    in0=xt,
                    scalar=1.0,
                    in1=xt,
                    op0=mybir.AluOpType.mult,
                    op1=mybir.AluOpType.mult,
                    accum_out=ssum,
                )
                # norm = sqrt(ssum); inv = 1/norm
                nrm = work.tile([P, 1], F32, tag="nrm")
                nc.scalar.activation(
                    nrm, ssum, mybir.ActivationFunctionType.Sqrt, bias=zero_bias
                )
                inv = work.tile([P, 1], F32, tag="inv")
                nc.vector.reciprocal(inv, nrm)

                # x_aug (bf16 copy of raw x)
                nc.vector.tensor_copy(x_aug[:, j, 0:D], xt)
                # xn = x * inv_norm  (bf16)
                xn = work.tile([P, D], BF16, tag="xn")
                nc.vector.tensor_scalar_mul(xn, xt, inv)
                # xbar transpose into xT[:, j] (contiguous destination)
                nc.sync.dma_start_transpose(xT[:, j], xn)

            # ---- Phase 2: QK upper parallelogram + exp ----
            for mi, (q0, qw) in enumerate(MEGAS):
                et = e_tiles[mi]
                j0 = q0 // P
                nj = qw // P
                for a in range(amax[mi] + 1):
                    ps = psum_s.tile([P, 512], F32, tag="psS")
                    for d in range(DT):
                        nc.tensor.matmul(
                            ps[:, :qw],
                            lhsT=xT[:, a, d, :],
                            rhs=xT[:, j0 : j0 + nj, d, :],
                            start=(d == 0),
                            stop=(d == DT - 1),
                        )
                    # E^T = exp(20*S - 20)
                    nc.scalar.activation(
                        et[:, a, :qw],
                        ps[:, :qw],
                        mybir.ActivationFunctionType.Exp,
                        bias=negtemp,
                        scale=TEMP_INV,
                    )

            def e_block(a, b):
                """AP of stored E^T[a-tile, b-tile cols] (only if a<=amax)."""
                mi = mega_of_block[b]
                q0 = MEGAS[mi][0]
                off = b * P - q0
                return e_tiles[mi][:, a, off : off + P]

            # ---- Phase 3: PV + blend per query block b ----
            for b in range(NT):
                mb = mega_of_block[b]
                # stage transposed stationaries (a > amax(mb)): one xbar
                # transpose per source mega (contiguous [128, qw'] slice)
                staged = {}
                a0 = amax[mb] + 1
                for mi in range(mb + 1, len(MEGAS)):
                    qm0, qmw = MEGAS[mi]
                    first = max(a0, qm0 // P)
                    last = amax[mi]
                    na = last - first + 1
                    if na <= 0:
                        continue
                    coff = first * P - qm0
                    src = e_tiles[mi][:, b, coff : coff + na * P]
                    stg = work.tile([P, 4, P], BF16, tag="etT", bufs=6)
                    nc.sync.dma_start_transpose(stg[:, :na, :], src)
                    for k in range(na):
                        staged[first + k] = stg[:, k, :]

                pv = psum_pv.tile([P, 1024], F32, tag="psPV")
                for a in range(NT):
                    lhsT = e_block(a, b) if a <= amax[mb] else staged[a]
                    nc.tensor.matmul(
                        pv[:, 0:512],
                        lhsT=lhsT,
                        rhs=x_aug[:, a, 0:512],
                        start=(a == 0),
                        stop=(a == NT - 1),
                    )
                    nc.tensor.matmul(
                        pv[:, 512 : D + 1],
                        lhsT=lhsT,
                        rhs=x_aug[:, a, 512 : D + 1],
                        start=(a == 0),
                        stop=(a == NT - 1),
                    )
                # blend: out = 0.5*x + (0.5/sum) * recal
                inv2 = work.tile([P, 1], F32, tag="inv2")
                nc.vector.reciprocal(inv2, pv[:, D : D + 1])
                invh = work.tile([P, 1], F32, tag="invh")
                nc.vector.tensor_scalar_mul(invh, inv2, 0.5)
                # t = recal * (0.5/sum) on ACT (idle during PV phase)
                t = work.tile([P, D], F32, tag="t")
                nc.scalar.mul(t, pv[:, 0:D], invh)
                ot = work.tile([P, D], F32, tag="ot")
                for h in range(2):
                    sl = slice(h * (D // 2), (h + 1) * (D // 2))
                    nc.vector.scalar_tensor_tensor(
                        ot[:, sl],
                        in0=x_full[:, b, sl],
                        scalar=0.5,
                        in1=t[:, sl],
                        op0=mybir.AluOpType.mult,
                        op1=mybir.AluOpType.add,
                    )
                    nc.gpsimd.dma_start(
                        out=out_dram[b * P : (b + 1) * P, sl], in_=ot[:, sl]
                    )

    if not nc.is_finalized():
        nc.finalize()
    return nc


def _get_program():
    if "nc" not in _CACHED:
        _CACHED["nc"] = build_program()
    return _CACHED["nc"]


def kernel(**inputs):
    features = inputs["features"]
    assert features.shape == (B, H, W, D), features.shape
    x = np.ascontiguousarray(features.reshape(B, N, D)).astype(np.float32)
    nc = _get_program()
    in_maps = [{"x": x[b]} for b in range(B)]
    res = run_bass_kernel_spmd(nc, in_maps, core_ids=list(range(B)))
    out = np.stack([res.results[b]["out"] for b in range(B)], axis=0)
    return out.reshape(B, H, W, D).astype(np.float32)


# revision 24
# speedup vs baseline: 1.1810x; 1.1810x over previous
"""CLIPtrase recalibration kernel for Trainium2 (Bass/Tile).

Per-batch computation (B=8, one batch element per NeuronCore):
    x  : (2304, 768) f32
    xn = x / ||x||_row
    S  = xn @ xn.T              (cosine correlation, symmetric)
    W  = softmax(S / 0.05, axis=-1)
    out = 0.5 * x + 0.5 * (W @ x)

Layout strategy (keys-on-partition, symmetric-S reuse):
    - xT: xn transposed into [d-partition, n-free] bf16 layout, built with PE
      transposes. Both QK operands come from xT.
    - S^T[a-keys(part), b-queries(free)] is computed per 512-query megablock
      m, but only for key tiles a <= amax(m) (upper parallelogram). exp is
      fused on ACT: E = exp(20*S - 20); the softmax row max is exactly
      cos(i,i)=1 (scaled: 20), so bias=-20 replaces max-subtraction. All E
      blocks stay resident in SBUF (bf16, ~49KB/partition).
    - E is symmetric, so the PV stationary blocks missing from the upper
      parallelogram are PE transposes (128 cols) of stored blocks instead of
      full recomputation (768 cols of matmul) -- a ~40% QK saving.
    - PV rhs = [x | 1] bf16: the ones column makes PSUM col 768 accumulate
      the softmax denominator alongside recal in cols 0:768.
"""

import sys

sys.path.insert(0, "/opt/trn_rl_repo")

import numpy as np

import concourse.bass as bass
import concourse.mybir as mybir
from concourse import bacc
from concourse.bass_utils import run_bass_kernel_spmd
from concourse.masks import make_identity
from concourse.tile import TileContext

F32 = mybir.dt.float32
BF16 = mybir.dt.bfloat16

B = 8
H = 48
W = 48
N = H * W          # 2304
D = 768
P = 128
NT = N // P        # 18 row tiles
DT = D // P        # 6 feature tiles
TEMP_INV = 20.0    # 1 / 0.05
# query megablocks: (start, width, n key tiles computed)
MEGAS = [(0, 512), (512, 512), (1024, 512), (1536, 512), (2048, 256)]

_CACHED = {}


def build_program():
    nc = bacc.Bacc()
    x_in = nc.declare_dram_parameter("x", [N, D], F32, isOutput=False)
    out_dram = nc.declare_dram_parameter("out", [N, D], F32, isOutput=True)

    # mega index and amax (last key-tile computed) per mega
    mega_of_block = {}
    amax = []
    for mi, (q0, qw) in enumerate(MEGAS):
        for b in range(q0 // P, (q0 + qw) // P):
            mega_of_block[b] = mi
        amax.append((q0 + qw) // P - 1)

    with TileContext(nc) as tc:
        with (
            tc.tile_pool(name="persist", bufs=1) as persist,
            tc.tile_pool(name="work", bufs=3) as work,
            tc.tile_pool(name="estore", bufs=1) as estore,
            tc.tile_pool(name="psS", bufs=2, space="PSUM") as psum_s,
            tc.tile_pool(name="psPV", bufs=2, space="PSUM") as psum_pv,
            tc.tile_pool(name="psT", bufs=2, space="PSUM") as psum_t,
        ):
            # persistent tensors
            x_full = persist.tile([P, NT, D], F32)      # raw x, resident
            xT = persist.tile([P, DT, N], BF16)         # xn^T  [d, n]
            x_aug = persist.tile([P, NT, D + 1], BF16)  # [x | 1] by row tile
            ident = persist.tile([P, P], BF16)
            make_identity(nc, ident)
            nc.vector.memset(x_aug[:, :, D : D + 1], 1.0)
            zero_bias = persist.tile([P, 1], F32)
            nc.vector.memset(zero_bias, 0.0)
            negtemp = persist.tile([P, 1], F32)
            nc.vector.memset(negtemp, -TEMP_INV)

            # E storage: per mega m, E^T[a, b-cols] for a <= amax(m)
            e_tiles = [
                estore.tile(
                    [P, amax[mi] + 1, qw], BF16, tag=f"E{mi}", name=f"E{mi}"
                )
                for mi, (q0, qw) in enumerate(MEGAS)
            ]

            # ---- Phase 1: load, row norms, xn, transpose to xT ----
            for j in range(NT):
                xt = x_full[:, j, :]
                # split DMAs across queues to cut arrival latency (the first
                # few tiles gate the first QK matmuls, so split those harder)
                nsplit = 4 if j < 4 else 2
                cs = D // nsplit
                for c in range(nsplit):
                    nc.sync.dma_start(
                        out=x_full[:, j, c * cs : (c + 1) * cs],
                        in_=x_in[j * P : (j + 1) * P, c * cs : (c + 1) * cs],
                    )

                # sum of squares in one pass: (x*1)*x with accumulate
                scratch = work.tile([P, D], BF16, tag="scratch")
                ssum = work.tile([P, 1], F32, tag="ssum")
                nc.vector.scalar_tensor_tensor(
                    scratch,
                    in0=xt,
                    scalar=1.0,
                    in1=xt,
                    op0=mybir.AluOpType.mult,
                    op1=mybir.AluOpType.mult,
                    accum_out=ssum,
                )
                # norm = sqrt(ssum); inv = 1/norm
                nrm = work.tile([P, 1], F32, tag="nrm")
                nc.scalar.activation(
                    nrm, ssum, mybir.ActivationFunctionType.Sqrt, bias=zero_bias
                )
                inv = work.tile([P, 1], F32, tag="inv")
                nc.vector.reciprocal(inv, nrm)

                # x_aug (bf16 copy of raw x)
                nc.vector.tensor_copy(x_aug[:, j, 0:D], xt)
                # xn = x * inv_norm  (bf16)
                xn = work.tile([P, D], BF16, tag="xn")
                nc.vector.tensor_scalar_mul(xn, xt, inv)
                # transpose xn into xT columns j*P:(j+1)*P
                for d in range(DT):
                    pt = psum_t.tile([P, P], BF16, tag="pt")
                    nc.tensor.transpose(pt, xn[:, d * P : (d + 1) * P], ident)
                    nc.vector.tensor_copy(xT[:, d, j * P : (j + 1) * P], pt)

            # ---- Phase 2: QK upper parallelogram + exp ----
            for mi, (q0, qw) in enumerate(MEGAS):
                et = e_tiles[mi]
                for a in range(amax[mi] + 1):
                    # skip columns left of the diagonal (blocks b < a are
                    # reconstructed from symmetry at PV time)
                    c0 = max(0, a * P - q0)
                    ps = psum_s.tile([P, 512], F32, tag="psS")
                    for d in range(DT):
                        nc.tensor.matmul(
                            ps[:, c0:qw],
                            lhsT=xT[:, d, a * P : (a + 1) * P],
                            rhs=xT[:, d, q0 + c0 : q0 + qw],
                            start=(d == 0),
                            stop=(d == DT - 1),
                        )
                    # E^T = exp(20*S - 20)
                    nc.scalar.activation(
                        et[:, a, c0:qw],
                        ps[:, c0:qw],
                        mybir.ActivationFunctionType.Exp,
                        bias=negtemp,
                        scale=TEMP_INV,
                    )

            def e_block(a, b):
                """AP of stored E^T[a-tile, b-tile cols] (only if a<=amax)."""
                mi = mega_of_block[b]
                q0 = MEGAS[mi][0]
                off = b * P - q0
                return e_tiles[mi][:, a, off : off + P]

            # ---- Phase 3: PV + blend per query block b ----
            for b in range(NT):
                mb = mega_of_block[b]
                # stationaries reconstructed by symmetry: beyond the
                # parallelogram (a > amax(mb)) plus the within-mega
                # sub-diagonal (a in mega(b), a > b). Groups of <=4
                # consecutive same-mega blocks: 4 PE transposes into one
                # PSUM bank, one DVE copy out.
                staged = {}
                miss = [
                    a
                    for a in range(NT)
                    if a > amax[mb] or (mega_of_block[a] == mb and a > b)
                ]
                groups = []
                cur = []
                for a in miss:
                    if cur and (
                        mega_of_block[a] != mega_of_block[cur[-1]]
                        or a != cur[-1] + 1
                        or len(cur) == 4
                    ):
                        groups.append(cur)
                        cur = []
                    cur.append(a)
                if cur:
                    groups.append(cur)
                for grp in groups:
                    ptb = psum_t.tile([P, 512], BF16, tag="pt")
                    for k, a in enumerate(grp):
                        nc.tensor.transpose(
                            ptb[:, k * P : (k + 1) * P], e_block(b, a), ident
                        )
                    stg = work.tile([P, 512], BF16, tag="etT", bufs=4)
                    nc.vector.tensor_copy(
                        stg[:, : len(grp) * P], ptb[:, : len(grp) * P]
                    )
                    for k, a in enumerate(grp):
                        staged[a] = stg[:, k * P : (k + 1) * P]

                pv = psum_pv.tile([P, 1024], F32, tag="psPV")
                for a in range(NT):
                    lhsT = staged[a] if a in staged else e_block(a, b)
                    nc.tensor.matmul(
                        pv[:, 0:512],
                        lhsT=lhsT,
                        rhs=x_aug[:, a, 0:512],
                        start=(a == 0),
                        stop=(a == NT - 1),
                    )
                    nc.tensor.matmul(
                        pv[:, 512 : D + 1],
                        lhsT=lhsT,
                        rhs=x_aug[:, a, 512 : D + 1],
                        start=(a == 0),
                        stop=(a == NT - 1),
                    )
                # blend: out = 0.5*x + (0.5/sum) * recal
                inv2 = work.tile([P, 1], F32, tag="inv2")
                nc.vector.reciprocal(inv2, pv[:, D : D + 1])
                invh = work.tile([P, 1], F32, tag="invh")
                nc.vector.tensor_scalar_mul(invh, inv2, 0.5)
                # t = recal * (0.5/sum) on ACT (idle during PV phase)
                t = work.tile([P, D], F32, tag="t")
                ot = work.tile([P, D], F32, tag="ot")
                for h in range(2):
                    sl = slice(h * (D // 2), (h + 1) * (D // 2))
                    nc.scalar.mul(t[:, sl], pv[:, sl], invh)
                    nc.vector.scalar_tensor_tensor(
                        ot[:, sl],
                        in0=x_full[:, b, sl],
                        scalar=0.5,
                        in1=t[:, sl],
                        op0=mybir.AluOpType.mult,
                        op1=mybir.AluOpType.add,
                    )
                    nc.sync.dma_start(
                        out=out_dram[b * P : (b + 1) * P, sl], in_=ot[:, sl]
                    )

    if not nc.is_finalized():
        nc.finalize()
    return nc


def _get_program():
    if "nc" not in _CACHED:
        _CACHED["nc"] = build_program()
    return _CACHED["nc"]


def kernel(**inputs):
    features = inputs["features"]
    assert features.shape == (B, H, W, D), features.shape
    x = np.ascontiguousarray(features.reshape(B, N, D)).astype(np.float32)
    nc = _get_program()
    in_maps = [{"x": x[b]} for b in range(B)]
    res = run_bass_kernel_spmd(nc, in_maps, core_ids=list(range(B)))
    out = np.stack([res.results[b]["out"] for b in range(B)], axis=0)
    return out.reshape(B, H, W, D).astype(np.float32)
